# revision 66
# baseline (speedup 1.0000x reference)
"""Linear-CKA map kernel for Trainium2 (8 NeuronCores, SPMD, no collectives).

Math: for activations X[l] ([B, D] per layer), the reference computes
Gram matrices G_l = X_l X_l^T, double-centers them (Gc = H G H), and
hsic[i,j] = <Gc_i, Gc_j>, cka = hsic / sqrt(diag outer).

We use the expansion (H idempotent, G symmetric):
    hsic_ij = S_ij - (2/B) * T_ij + u_i u_j / B^2
      S_ij = <G_i, G_j>
      rowsum_l[b] = sum_c G_l[b, c] = X_l[b, :] . s_l,  s_l = sum_b X_l[b, :]
      T_ij = sum_b rowsum_i[b] rowsum_j[b]
      u_l  = s_l . s_l

Sharding: the Gram is symmetric over 16x16 blocks of [128, 128]; every
unordered block pair must be co-resident on some core.  A covering
design with 7 column-blocks per core (the information-theoretic floor
at this granularity: 6-block coverings do not exist, by a Fisher /
intersecting-family argument) brings the per-core HBM read down to
L * D * 896 fp8 bytes = 44 MB, vs 63 MB for the baseline 10-block
cyclic cover.  All cores run the SAME program over 7 SBUF "slots"; a
per-core slot permutation SIGMA (found by annealing over an ILP cover)
maps slots to physical blocks so that the fixed slot-pair work list
covers all 136 physical pairs:

  slot-pairs = 2 self pairs (0,0),(1,1) + all 18 cross pairs (a,b)
  with hub a <= 3 -- four "fans" with contiguous partner ranges so
  each PSUM bank hosts exactly one accumulation group:
     F0 = (0 x slots 0..6)   896 cols   2 banks
     F1 = (1 x slots 1..6)   768 cols   2 banks
     F2 = (2 x slots 3..6)   512 cols   1 bank
     F3 = (3 x slots 4..6)   384 cols   1 bank
  The blocks at slots 0,1 across the 8 cores partition all 16 blocks,
  so the two self pairs compute each Gram diagonal block exactly once.

Per layer the fans run as four sequential phase groups (F0, F2, F1,
F3); each group's PSUM banks drain while the following groups compute,
so the next layer's start=True matmuls find their banks free.  pf0 is
layer-double-buffered (the only drain that would otherwise gate the
next layer's first matmul) and the drains are split between ScalarE
(pf0, pf2, pf3) and VectorE (pf1) with all readers of any one PSUM
tile kept on a single engine -- the Tile framework keeps one accessor
chain per tile, so mixed-engine readers would serialize through
cross-engine semaphore hops and pace the whole loop below the DMA
rate.  Gram blocks are stored interleaved [b, c, layer] in fp8
(off-diag at 0.5 scale, diag at 1/16 scale to stay inside e4m3
range), one SBUF tile per (fan, engine) to avoid false WAW chains.
A TensorE tail then reduces each stored slot-pair against itself over
layers ([128b, 2i, 4c x 24l] DoubleRow oct matmuls) into [96, 96]
tiles -- rotating over the freed pf2/pf3/pf1 banks, drained to two
per-engine staging tiles and streamed out in chunks -- whose
quad-diagonal 24x24 blocks the host extracts.  The host de-duplicates
redundantly covered pairs with a precomputed (core, slot-pair)
ownership map and adds the O(L*B*D) row-sum statistics T and u.  No
device collective at all.  TimelineSim: 189.2us (baseline) ->
140.4us; DMA roofline for the 896-column read is ~124us.
"""

import numpy as np
import ml_dtypes

L, B, D = 24, 2048, 2048
NCORES = 8
P = 128
NS = 7                      # column-block slots per core
JT = 8                      # 256-deep DoubleRow contraction tiles
JG = 2                      # j-tiles fetched per rhs DMA
W = NS * P                  # 896 packed columns per core

# fans: (hub slot, first partner slot, #partners); bank-aligned matmul
# splits of each fan's PSUM tile are derived below
FANS = [(0, 0, 7), (1, 1, 6), (2, 3, 4), (3, 4, 3)]
# cross slot-pairs in g8 storage order: F0 partners 1..6, F3, F1
# partners 2..6, F2 (the hub-0/hub-1 self pairs live in gd0/gd1)
CROSSQ = (
    [(0, b) for b in range(1, 7)]
    + [(3, b) for b in range(4, 7)]
    + [(1, b) for b in range(2, 7)]
    + [(2, b) for b in range(3, 7)]
)
NQ = len(CROSSQ)            # 18
NPAIR = NQ + 2              # + the two self pairs
# S-stage processing order: pairs sorted by when their source tile's last
# layer-23 drain lands (g8f2 first, then the diag stores, g8f0, g8f1, and
# g8f3 last), so the S-stage starts right at the end of the Gram loop
QORDER = [1, 16, 17, 18, 19, 0, 2, 3, 4, 5, 6, 7, 11, 12, 13, 14, 15, 8, 9, 10]

# SIGMA[k][s] = physical block held in slot s on core k (annealed so the
# fixed slot-pair list covers all 136 block pairs and slots {0,1}
# partition the 16 diagonal blocks)
SIGMA = [
    [15, 8, 12, 11, 1, 7, 6],
    [2, 11, 14, 8, 9, 10, 3],
    [4, 13, 5, 0, 8, 12, 11],
    [3, 10, 13, 9, 12, 7, 15],
    [0, 7, 6, 5, 10, 3, 9],
    [5, 14, 0, 2, 15, 7, 1],
    [1, 9, 4, 10, 7, 3, 15],
    [12, 6, 13, 4, 1, 2, 14],
]

# host-side dedup: first core covering a physical pair owns it
_OWNER_W = np.zeros((NCORES, NQ), dtype=np.float64)
_seen = {}
for _k in range(NCORES):
    for _q, (_a, _b) in enumerate(CROSSQ):
        _u, _v = SIGMA[_k][_a], SIGMA[_k][_b]
        _pp = (min(_u, _v), max(_u, _v))
        if _pp not in _seen:
            _seen[_pp] = True
            # weight 2 for Gram symmetry x4 to undo the 0.5 fp8 store scale
            _OWNER_W[_k][_q] = 8.0
assert len(_seen) == 120

_NC_CACHE = {}


def _build():
    if "nc" in _NC_CACHE:
        return _NC_CACHE["nc"]
    import concourse.bass as bass
    from concourse import bacc, mybir, tile

    f32 = mybir.dt.float32
    bf16 = mybir.dt.bfloat16
    fp8 = mybir.dt.float8e4
    DR = mybir.MatmulPerfMode.DoubleRow

    nc = bacc.Bacc("TRN2", target_bir_lowering=False, debug=False)

    # host-packed to match the SBUF staging tiles: one fully linear
    # [P, JG, 2, W] block per (layer, jg) DMA
    xr = nc.dram_tensor("xr", [L, P, JT // JG, JG, 2, W], fp8, kind="ExternalInput")
    s_out = nc.dram_tensor("s_out", [4 * L, NPAIR * 4 * L], f32, kind="ExternalOutput")

    QW = 4 * L  # 96: quad/oct S-stage operand width (4 Gram cols x L layers)

    with tile.TileContext(nc) as tc:
        with (
            tc.tile_pool(name="gb", bufs=1) as gbpool,
            tc.tile_pool(name="rt", bufs=4) as rtpool,
            tc.tile_pool(name="psum", bufs=1, space=bass.MemorySpace.PSUM) as pfpool,
            tc.tile_pool(name="psum0", bufs=2, space=bass.MemorySpace.PSUM) as pf0pool,
        ):
            # persistent SBUF Gram store, interleaved [b, c, layer].  One
            # tile per fan destination: Tile tracks WAW at tile granularity,
            # so a single shared store would serialize the ScalarE and
            # VectorE drain chains against each other across layers.
            # diag blocks store fp8 at 1/16 scale (|G_bb| <= ~2370 -> 148,
            # inside even the inf-style e4m3 range); the S-stage then runs
            # DoubleRow octs for every pair, the host undoes the scale
            gd0 = gbpool.tile([P, P, L], fp8, tag="gd0", name="gd0")
            gd1 = gbpool.tile([P, P, L], fp8, tag="gd1", name="gd1")
            g8f0 = gbpool.tile([P, 6 * P, L], fp8, tag="g8f0", name="g8f0")
            g8f1 = gbpool.tile([P, 5 * P, L], fp8, tag="g8f1", name="g8f1")
            g8f2 = gbpool.tile([P, 4 * P, L], fp8, tag="g8f2", name="g8f2")
            g8f3 = gbpool.tile([P, 3 * P, L], fp8, tag="g8f3", name="g8f3")
            # S-stage staging, one per drain engine (same tile-WAW issue);
            # the S accumulators rotate (pf2, pf3, pf1): pf2/pf3 drain on
            # ScalarE into sSa, pf1 on VectorE into sSb
            NSA = NPAIR - NPAIR // 3
            NSB = NPAIR // 3
            sSa = gbpool.tile([QW, NSA * QW], f32, tag="sSa", name="sSa")
            sSb = gbpool.tile([QW, NSB * QW], f32, tag="sSb", name="sSb")

            # one PSUM tile per fan; a fan's matmuls split on its tile's
            # bank boundaries so each bank hosts ONE accumulation group.
            # pf0 is layer-double-buffered (it is the only fan whose drain
            # would otherwise gate the next layer's first matmul); the
            # S-stage later reuses pf2/pf3's banks as its accumulators.
            pf1 = pfpool.tile([P, 6 * P], f32, tag="pf1", name="pf1")
            pf2 = pfpool.tile([P, 4 * P], f32, tag="pf2", name="pf2")
            pf3 = pfpool.tile([P, 3 * P], f32, tag="pf3", name="pf3")

            def fan_matmuls(rt, jj, st, sp, pf, hub, p0, np_):
                lhs = rt[:, jj, :, hub * P : (hub + 1) * P]
                # split the fan's [p0, p0+np_) partner range on the PSUM
                # bank (512 f32) boundaries of pf
                c = 0
                while c < np_ * P:
                    cw = min(512 - c % 512, np_ * P - c)
                    nc.tensor.matmul(
                        pf[:, c : c + cw],
                        lhsT=lhs,
                        rhs=rt[:, jj, :, p0 * P + c : p0 * P + c + cw],
                        start=st,
                        stop=sp,
                        perf_mode=DR,
                    )
                    c += cw

            # per layer the four fans run as four sequential phase groups;
            # each group's PSUM banks drain (ScalarE/VectorE split) while the
            # following three groups compute, so the next layer's start=True
            # on the same banks always finds them free
            # All readers of one PSUM tile stay on ONE engine: Tile keeps a
            # single accessor chain per tile, so mixed-engine readers of the
            # same tile serialize with a cross-engine semaphore hop per
            # reader.  pf0/pf2 drain on ScalarE, pf1/pf3 on VectorE.
            def copies0(l, pf0):
                nc.scalar.mul(gd0[:, :, l], pf0[:, 0:P], 0.0625)
                nc.scalar.mul(g8f0[:, :, l], pf0[:, P : 7 * P], 0.5)

            def copies1(l, pf):
                nc.vector.tensor_scalar_mul(gd1[:, :, l], pf[:, 0:P], 0.0625)
                nc.vector.tensor_scalar_mul(g8f1[:, :, l], pf[:, P : 6 * P], 0.5)

            def copies2(l, pf):
                nc.scalar.mul(g8f2[:, :, l], pf[:, :], 0.5)

            def copies3(l, pf):
                # the last layer's pf3 drain rides VectorE instead: the
                # tail's critical path is the serial ScalarE drain chain of
                # layer L-1 (via the framework's tick waits), and one
                # cross-engine accessor hop on the pf3/g8f3 tiles is cheaper
                # than 505ns of extra chain (measured: applying this to L-2
                # as well lengthens that layer's VectorE chain and loses)
                if l >= L - 1:
                    nc.vector.tensor_scalar_mul(g8f3[:, :, l], pf[:, :], 0.5)
                else:
                    nc.scalar.mul(g8f3[:, :, l], pf[:, :], 0.5)

            for l in range(L):
                # one whole-layer DMA for all but the last layer (same bytes,
                # 69 fewer issue/semaphore events); layer L-1 keeps per-jg
                # DMAs so its early j-tiles are available as they land
                if l < L - 1:
                    rtf = rtpool.tile(
                        [P, JT // JG, JG, 2, W], fp8, tag="rtL", name="rtf"
                    )
                    nc.sync.dma_start(rtf[:, :, :, :, :], xr[l])
                    rts = [rtf[:, jg] for jg in range(JT // JG)]
                else:
                    rts = []
                    for jg in range(JT // JG):
                        rt = rtpool.tile([P, JG, 2, W], fp8, tag="rt", name="rt")
                        nc.sync.dma_start(rt[:, :, :, :], xr[l, :, jg])
                        rts.append(rt)
                pf0 = pf0pool.tile([P, 7 * P], f32, tag="pf0", name="pf0")
                # group order puts each drain as far as possible ahead of
                # the next layer's reuse of its banks: pf0 (double-buffered,
                # drained lazily after pf2's), then pf2/pf3/pf1 whose bank
                # reuse comes 1-3 groups into the next layer
                groups = (
                    (pf0, 0, 0, 7, None),
                    (pf2, 2, 3, 4, copies2),
                    (pf1, 1, 1, 6, copies1),
                    (pf3, 3, 4, 3, copies3),
                )
                # The last layer splits its j-loop: every group's jg0..jg2
                # matmuls run while the final DMA tile is still in flight,
                # so only the short jg3 chunks (and the drains) remain after
                # the last tile's semaphore fires -- pulling the whole
                # S-stage tail ~3us earlier.  Mid-loop layers keep the
                # group-sequential order that paces the drain pipeline.
                jg_hi = JT // JG if l < L - 1 else JT // JG - 1
                for pf, hub, p0, np_, copies in groups:
                    for jg in range(jg_hi):
                        for jj in range(JG):
                            j = jg * JG + jj
                            fan_matmuls(
                                rts[jg], jj, j == 0, j == JT - 1, pf, hub, p0, np_
                            )
                    if l < L - 1:
                        if copies is not None:
                            copies(l, pf)
                            if pf is pf2:
                                copies0(l, pf0)
                if l == L - 1:
                    for pf, hub, p0, np_, copies in groups:
                        jg = JT // JG - 1
                        for jj in range(JG):
                            j = jg * JG + jj
                            fan_matmuls(
                                rts[jg], jj, False, j == JT - 1, pf, hub, p0, np_
                            )
                        if copies is not None:
                            copies(l, pf)
                            if pf is pf2:
                                copies0(l, pf0)

            # S-stage tail: reduce each stored slot-pair over (b, c) into
            # a [QW, QW] PSUM tile (4 Gram columns per matmul; the host
            # keeps the quad-diagonal [L, L] blocks).  Self pairs run in
            # bf16 quads, cross pairs in fp8 DoubleRow octs.
            # cross pair qi-2 -> (fan tile, local block) in CROSSQ order
            qsrc = (
                [(g8f0, i) for i in range(6)]
                + [(g8f3, i) for i in range(3)]
                + [(g8f1, i) for i in range(5)]
                + [(g8f2, i) for i in range(4)]
            )
            for pos in range(NPAIR):
                qi = QORDER[pos]
                # rotate through the freed pf2/pf3/pf1 banks; pf2/pf3's
                # readers stay on ScalarE and pf1's on VectorE throughout
                pt = (pf2, pf3, pf1)[pos % 3][0:QW, 0:QW]
                if True:
                    if qi < 2:
                        gt, base = (gd0, gd1)[qi], 0
                    else:
                        gt, lq = qsrc[qi - 2]
                        base = lq * P
                    for ci, c in enumerate(range(0, P, 8)):
                        nc.tensor.matmul(
                            pt[:, :],
                            lhsT=gt[:, base + c : base + c + 8, :].rearrange(
                                "p (i x) l -> p i (x l)", i=2
                            ),
                            rhs=gt[:, base + c : base + c + 8, :].rearrange(
                                "p (i x) l -> p i x l", i=2
                            ),
                            start=(ci == 0),
                            stop=(c + 8 == P),
                            perf_mode=DR,
                        )
                # alternate drain engines (each with its own staging tile)
                # so the psS ping-pong round trip halves; stream the export
                # in chunks so the final DMA only covers the last few pairs
                if pos % 3 != 2:
                    h = pos - pos // 3
                    nc.scalar.copy(sSa[:, h * QW : (h + 1) * QW], pt[:, :])
                else:
                    h = pos // 3
                    nc.vector.tensor_copy(sSb[:, h * QW : (h + 1) * QW], pt[:, :])
                if pos == 10:
                    # sSa slots 0..7 are final
                    nc.sync.dma_start(s_out[:, : 8 * QW], sSa[:, : 8 * QW])
                elif pos == 16:
                    # sSa slots 8..11 are final
                    nc.sync.dma_start(
                        s_out[:, 8 * QW : 12 * QW], sSa[:, 8 * QW : 12 * QW]
                    )
                elif pos == 17:
                    # last VectorE pair completes sSb
                    nc.sync.dma_start(s_out[:, NSA * QW :], sSb[:, :])
            nc.sync.dma_start(s_out[:, 12 * QW : NSA * QW], sSa[:, 12 * QW :])

    nc.compile()
    _NC_CACHE["nc"] = nc
    return nc


def _run(activations, trace=False):
    from concourse.bass_utils import run_bass_kernel_spmd

    x = np.asarray(activations, dtype=np.float32)
    assert x.shape == (L, B, D)
    xt_np = np.ascontiguousarray(x.transpose(0, 2, 1)).astype(ml_dtypes.float8_e4m3)
    s_star = xt_np.astype(np.float64).sum(axis=2)  # [L, D], exact sum of fp8 X

    in_maps = []
    for k in range(NCORES):
        cols = np.concatenate(
            [xt_np[:, :, blk * P : (blk + 1) * P] for blk in SIGMA[k]], axis=2
        )  # [L, D, W]
        # pack to the SBUF staging layout: [L, p, jg, jj, i, w] so a
        # whole-layer DMA is one fully contiguous block per partition
        packed = np.ascontiguousarray(
            cols.reshape(L, JT // JG, JG, 2, P, W).transpose(0, 4, 1, 2, 3, 5)
        )
        in_maps.append({"xr": packed})
    nc = _build()
    try:
        res = run_bass_kernel_spmd(
            nc, in_maps, core_ids=list(range(NCORES)), trace=trace
        )
    except Exception:
        # transient NRT_EXEC_UNIT_UNRECOVERABLE device states have been
        # observed to clear on the next attempt
        import time

        time.sleep(5)
        res = run_bass_kernel_spmd(
            nc, in_maps, core_ids=list(range(NCORES)), trace=trace
        )

    # export slot layout (positional in QORDER): slots 0..13 = positions
    # with pos % 3 != 2 (ScalarE staging), 14..19 = the rest (VectorE)
    _NSA = NPAIR - NPAIR // 3

    def _slot(qi):
        pos = QORDER.index(qi)
        return pos - pos // 3 if pos % 3 != 2 else _NSA + pos // 3

    S = np.zeros((L, L), dtype=np.float64)
    for k in range(NCORES):
        # [QW, NPAIR, QW] -> per pair sum the quad-diagonal [L, L] blocks
        g = res.results[k]["s_out"].astype(np.float64).reshape(4, L, NPAIR, 4, L)
        gd = np.einsum("dicdj->cij", g)  # [slot, L, L]
        S += 256.0 * (gd[_slot(0)] + gd[_slot(1)])  # self pairs: 1/16 scale
        for q in range(NQ):
            if _OWNER_W[k][q]:
                S += _OWNER_W[k][q] * gd[_slot(2 + q)]

    # row-sum statistics are O(L*B*D) -- computed host-side on the same
    # quantized values the device consumed
    xq = xt_np.astype(np.float32)                  # [L, D, B]
    rowsum = np.einsum("ldb,ld->lb", xq, s_star.astype(np.float32))
    T = np.einsum("ib,jb->ij", rowsum, rowsum, dtype=np.float64)
    u = np.einsum("ld,ld->l", s_star, s_star)
    hsic = S - (2.0 / B) * T + np.outer(u, u) / (B * B)
    norms = np.sqrt(np.diagonal(hsic))
    cka = hsic / (norms[:, None] * norms[None, :])
    return cka.astype(np.float32), res


def kernel(activations):
    cka, _ = _run(activations, trace=False)
    return cka


def run_traced(activations):
    return _run(activations, trace=True)
